# revision 1
# baseline (speedup 1.0000x reference)
"""Single-head causal attention with RoPE on 8 TRN2 NeuronCores.

Problem: B=4, T=4096, C=2048, D=128 (fp32 reference).
  q/k/v = x @ W{q,k,v}.T + b ; rope(q), rope(k); causal softmax(q k^T / sqrt(D)) @ v

Sharding: core c -> batch b = c//2, sequence-half h = c%2 with a zig-zag
(load-balanced) split of query rows: h=0 owns 512-row tiles {0,1,6,7}
(rows [0,1024) u [3072,4096)), h=1 owns tiles {2,3,4,5} (rows [1024,3072)).
Both halves do exactly 72 score-tile matmuls + 264 AV matmuls, so the causal
work is perfectly balanced. K/V are computed for the full sequence on both
cores of a pair (duplicated; no collectives needed).

One SPMD graph for all 8 cores; the causal structure difference between the
two halves is handled by a runtime If() on a per-core flag input.

Kernel math (all matmuls in bf16, fp32 PSUM accumulation / softmax):
  - Q^T/K^T/V^T projections with T on the moving dim (N=512) for PE efficiency
  - scores computed transposed, S^T[k, q], so softmax needs no transposes
  - exp without max-subtraction (logits are ~N(0,0.8); exp stays tiny in f32)
  - causal mask = multiplicative {0,1} tile AFTER exp (only 4 static patterns)
  - softmax denominator folded into the AV matmul by appending a ones column
    to V (V_aug [k, 129]); PSUM accumulates [AV | rowsum] in natural [q,d]
    layout, then a per-partition reciprocal-scale normalizes.
"""

import sys

if "/opt/trn_rl_repo" not in sys.path:
    sys.path.insert(0, "/opt/trn_rl_repo")

import numpy as np
import ml_dtypes

import concourse.mybir as mybir
import concourse.tile as tile
from concourse import bacc
from concourse.masks import make_identity
from concourse.bass_utils import run_bass_kernel_spmd

BF16 = mybir.dt.bfloat16
F32 = mybir.dt.float32
P = 128
B, T, C, D = 4, 4096, 2048, 128
CC = C // P          # 16 contraction chunks
TQ = T // 2          # 2048 own query rows per core
NT = T // 512        # 8 sequence tiles
NQ = TQ // 512       # 4 own query tiles
KC = T // P          # 32 key chunks
SCALE = float(D) ** -0.5
ROPE_BASE = 10000.0

# zig-zag query-tile ownership (global 512-row tile indices)
JOBS_H0 = (0, 3, 4, 7)
JOBS_H1 = (1, 2, 5, 6)

_NC_CACHE = None


def _build_nc():
    nc = bacc.Bacc("TRN2", target_bir_lowering=False, debug=False, num_devices=8)

    xT = nc.declare_dram_parameter("xT", [C, T], BF16, isOutput=False)
    wqP = nc.declare_dram_parameter("wqP", [P, CC * D], BF16, isOutput=False)
    wkP = nc.declare_dram_parameter("wkP", [P, CC * D], BF16, isOutput=False)
    wvP = nc.declare_dram_parameter("wvP", [P, CC * D], BF16, isOutput=False)
    cosT = nc.declare_dram_parameter("cosT", [D, T], BF16, isOutput=False)
    sinT = nc.declare_dram_parameter("sinT", [D, T], BF16, isOutput=False)
    bq = nc.declare_dram_parameter("bq", [D, 1], F32, isOutput=False)
    bk = nc.declare_dram_parameter("bk", [D, 1], F32, isOutput=False)
    bv = nc.declare_dram_parameter("bv", [D, 1], F32, isOutput=False)
    tri = nc.declare_dram_parameter("tri", [4, P, 512], BF16, isOutput=False)
    flag = nc.declare_dram_parameter("flag", [1, 1], mybir.dt.int32, isOutput=False)
    out = nc.declare_dram_parameter("out", [TQ, D], F32, isOutput=True)

    with tile.TileContext(nc) as tc:
        with (
            tc.tile_pool(name="big", bufs=1) as big,
            tc.tile_pool(name="xin", bufs=5) as xin,
            tc.tile_pool(name="work", bufs=6) as work,
            tc.tile_pool(name="outp", bufs=6) as outp,
            tc.tile_pool(name="ps", bufs=4, space="PSUM") as ps,
            tc.tile_pool(name="acc", bufs=1, space="PSUM") as accp,
        ):
            # ---- constants / small inputs ----
            wq_sb = big.tile([P, CC, D], BF16, name="wq_sb")
            wk_sb = big.tile([P, CC, D], BF16, name="wk_sb")
            wv_sb = big.tile([P, CC, D], BF16, name="wv_sb")
            flag_sb = big.tile([1, 1], mybir.dt.int32, name="flag_sb")
            nc.scalar.dma_start(flag_sb[:], flag[:])
            bq_sb = big.tile([P, 1], F32, name="bq_sb")
            bk_sb = big.tile([P, 1], F32, name="bk_sb")
            bv_sb = big.tile([P, 1], F32, name="bv_sb")

            # ---- persistent activations (per-512-slice tiles so reads
            # depend only on their own slice's writers) ----
            kTs = [big.tile([P, 512], BF16, tag=f"kT{t}", name=f"kT{t}")
                   for t in range(NT)]
            qTs = [big.tile([P, 512], BF16, tag=f"qT{t}", name=f"qT{t}")
                   for t in range(NT)]
            vAs = [big.tile([P, 4, D + 1], BF16, tag=f"vA{t}", name=f"vA{t}")
                   for t in range(NT)]
            cos_sb = big.tile([P, T], BF16, name="cos_sb")
            sin_sb = big.tile([P, T], BF16, name="sin_sb")
            tri_sb = big.tile([P, 4, 512], BF16, name="tri_sb")
            ident = big.tile([P, P], BF16, name="ident")


            xT_r = xT.rearrange("(cc p) t -> p cc t", p=P)
            H = D // 2

            def proj(w_sb, b_sb, xt):
                pp = ps.tile([P, 512], F32, tag="ps")
                for cc in range(CC):
                    nc.tensor.matmul(pp[:], w_sb[:, cc], xt[:, cc],
                                     start=(cc == 0), stop=(cc == CC - 1))
                raw = work.tile([P, 512], BF16, tag="prj")
                nc.vector.tensor_scalar_add(raw[:], pp[:], b_sb[:])
                return raw

            def rope(dst, raw, sl):
                tmp = work.tile([P, 512], BF16, tag="rtmp")
                nc.scalar.dma_start(tmp[0:H, :], raw[H:P, :])
                nc.scalar.dma_start(tmp[H:P, :], raw[0:H, :])
                nc.vector.tensor_mul(tmp[:], tmp[:], sin_sb[:, sl])
                nc.vector.tensor_mul(dst[:], raw[:], cos_sb[:, sl])
                nc.vector.tensor_add(dst[:], dst[:], tmp[:])

            def attention_job(tj, ol0):
                ql0 = tj * 512             # global column offset in qT
                kc_max = 4 * (tj + 1)      # causal key-chunk bound
                accs = [accp.tile([P, D + 1], F32, tag=f"acc{j}",
                                  name=f"acc_{tj}_{j}")[:]
                        for j in range(4)]
                for kc in range(kc_max):
                    ps_s = ps.tile([P, 512], F32, tag="ps")
                    nc.tensor.matmul(ps_s[:],
                                     kTs[kc // 4][:, (kc % 4) * P:
                                                  (kc % 4 + 1) * P],
                                     qTs[tj][:],
                                     start=True, stop=True)
                    ex = work.tile([P, 512], BF16, tag="expP")
                    nc.scalar.activation(ex[:], ps_s[:],
                                         mybir.ActivationFunctionType.Exp,
                                         scale=SCALE)
                    doff = kc - 4 * tj
                    if 0 <= doff < 4:
                        nc.vector.tensor_mul(ex[:], ex[:], tri_sb[:, doff])
                    for j in range(4):
                        kc_max_j = 4 * tj + j + 1
                        if kc < kc_max_j:
                            nc.tensor.matmul(
                                accs[j], ex[:, j * P:(j + 1) * P],
                                vAs[kc // 4][:, kc % 4],
                                start=(kc == 0), stop=(kc == kc_max_j - 1))
                for j in range(4):
                    rcp = outp.tile([P, 1], F32, tag="rcp")
                    nc.vector.reciprocal(rcp[:], accs[j][:, D:D + 1])
                    ob = outp.tile([P, D], F32, tag="ob")
                    nc.vector.tensor_scalar_mul(ob[:], accs[j][:, 0:D], rcp[:])
                    nc.sync.dma_start(
                        out[ol0 + j * P: ol0 + (j + 1) * P, :], ob[:])

            def branch(jobs):
                asc = sorted(jobs)
                # per-branch constant loads (the If-entry barrier waits on all
                # pre-emitted instructions, so keep everything inside)
                nc.scalar.dma_start(wk_sb[:],
                                    wkP.rearrange("p (cc d) -> p cc d", d=D))
                nc.scalar.dma_start(wv_sb[:],
                                    wvP.rearrange("p (cc d) -> p cc d", d=D))
                nc.scalar.dma_start(wq_sb[:],
                                    wqP.rearrange("p (cc d) -> p cc d", d=D))
                nc.scalar.dma_start(bk_sb[:], bk[:])
                nc.scalar.dma_start(bv_sb[:], bv[:])
                nc.scalar.dma_start(bq_sb[:], bq[:])
                nc.scalar.dma_start(cos_sb[:], cosT[:])
                nc.scalar.dma_start(sin_sb[:], sinT[:])
                nc.scalar.dma_start(tri_sb[:], tri.rearrange("j p q -> p j q"))
                make_identity(nc, ident[:])
                for t in range(NT):
                    nc.vector.memset(vAs[t][:, :, D], 1.0)
                # K/V projection + rope + V_aug stream over the full sequence;
                # Q only for this branch's own query tiles.
                for tt in range(NT):
                    sl = slice(tt * 512, (tt + 1) * 512)
                    xt = xin.tile([P, CC, 512], BF16, tag="xin")
                    gw = 1 if tt == 0 else 4   # fine-grained first tile
                    for g in range(CC // gw):
                        nc.sync.dma_start(xt[:, gw * g:gw * (g + 1), :],
                                          xT_r[:, gw * g:gw * (g + 1), sl])
                    kraw = proj(wk_sb, bk_sb, xt)
                    rope(kTs[tt], kraw, sl)
                    if tt in jobs:
                        qraw = proj(wq_sb, bq_sb, xt)
                        rope(qTs[tt], qraw, sl)
                    vraw = proj(wv_sb, bv_sb, xt)
                    for kk in range(4):
                        ps_t = ps.tile([P, P], BF16, tag="ps")
                        nc.tensor.transpose(ps_t[:],
                                            vraw[:, kk * P:(kk + 1) * P],
                                            ident[:])
                        nc.scalar.copy(vAs[tt][:, kk, 0:D], ps_t[:])
                    # interleave attention jobs as their K prefixes complete
                    for tj in jobs:
                        if 4 * (tj + 1) == 4 * (tt + 1) and tj <= tt:
                            pass  # handled below via ready list
                # emit attention jobs ascending so early jobs only depend on
                # early K tiles and overlap the stream tail
                for tj in asc:
                    attention_job(tj, asc.index(tj) * 512)

            fv = nc.values_load(flag_sb[0:1, 0:1].to_broadcast((1, 1)))
            with tc.If(fv < 1) as cmp:
                branch(JOBS_H0)
            with cmp.Else():
                branch(JOBS_H1)

    nc.compile()
    return nc


def _get_nc():
    global _NC_CACHE
    if _NC_CACHE is None:
        _NC_CACHE = _build_nc()
    return _NC_CACHE


def _own_rows(h):
    if h == 0:
        return np.r_[0:512, 1536:2560, 3584:4096]
    return np.r_[512:1536, 2560:3584]


def _prep_in_maps(x, Wq, Wk, Wv, bq, bk, bv):
    x = np.asarray(x, np.float32)
    bf = ml_dtypes.bfloat16

    # rope tables (rotate-half convention), pre-signed sin
    half = D // 2
    inv = 1.0 / (ROPE_BASE ** (np.arange(half, dtype=np.float32) / half))
    ang = np.arange(T, dtype=np.float32)[:, None] * inv[None, :]       # [T, 64]
    cos_full = np.concatenate([np.cos(ang), np.cos(ang)], 1).T         # [128, T]
    sin_full = np.concatenate([-np.sin(ang), np.sin(ang)], 1).T        # [128, T]
    cosT = cos_full.astype(bf)
    sinT = sin_full.astype(bf)

    # 4 diagonal mask patterns: tri[j][k, q] = 1 if k + 128*j <= q
    k_idx = np.arange(P)[:, None]
    q_idx = np.arange(512)[None, :]
    tri = np.stack([(k_idx + P * j <= q_idx) for j in range(4)]).astype(bf)

    def _wP(W):
        # [D, C] -> [C, D] -> [p, cc, d] -> [P, CC*D]
        wT = np.asarray(W, np.float32).T.reshape(CC, P, D).transpose(1, 0, 2)
        return np.ascontiguousarray(wT.reshape(P, CC * D)).astype(bf)

    wqP, wkP, wvP = _wP(Wq), _wP(Wk), _wP(Wv)
    bq_a = np.ascontiguousarray(np.asarray(bq, np.float32).reshape(D, 1))
    bk_a = np.ascontiguousarray(np.asarray(bk, np.float32).reshape(D, 1))
    bv_a = np.ascontiguousarray(np.asarray(bv, np.float32).reshape(D, 1))

    xT_cache = {}
    in_maps = []
    for c in range(8):
        b, h = c // 2, c % 2
        if b not in xT_cache:
            xT_cache[b] = np.ascontiguousarray(x[b].T).astype(bf)  # [C, T]
        xT_b = xT_cache[b]
        in_maps.append({
            "xT": xT_b,
            "wqP": wqP, "wkP": wkP, "wvP": wvP,
            "cosT": cosT, "sinT": sinT,
            "bq": bq_a, "bk": bk_a, "bv": bv_a,
            "tri": tri,
            "flag": np.array([[h]], np.int32),
        })

    return in_maps


def kernel(x, Wq, Wk, Wv, bq, bk, bv):
    nc = _get_nc()
    in_maps = _prep_in_maps(x, Wq, Wk, Wv, bq, bk, bv)
    res = run_bass_kernel_spmd(nc, in_maps, core_ids=list(range(8)))

    out = np.empty((B, T, D), np.float32)
    for c in range(8):
        b, h = c // 2, c % 2
        out[b, _own_rows(h)] = res.results[c]["out"]
    return out



# revision 4
# speedup vs baseline: 1.1365x; 1.1365x over previous
"""Single-head causal attention with RoPE on 8 TRN2 NeuronCores.

Problem: B=4, T=4096, C=2048, D=128 (fp32 reference).
  q/k/v = x @ W{q,k,v}.T + b ; rope(q), rope(k); causal softmax(q k^T / sqrt(D)) @ v

Sharding: core c -> batch b = c//2, sequence-half h = c%2 with a load-balanced
split of 512-row query tiles: h=0 owns tiles {0,2,4,7}, h=1 owns {1,3,5,6}.
h=1 needs K/V only for tiles 0..6, so it skips tile 7's x load and K/V
projection entirely; h=0 carries 8 K/V tiles but 4 fewer score chunks, which
roughly balances the cores. K/V are computed for the needed prefix on both
cores of a pair (duplicated; no collectives, so each core simulates alone).

One SPMD graph for all 8 cores; the structural difference between the two
halves is a runtime If() on a per-core flag input.

Kernel math:
  - Projections run on fp8 (float8e4) with DoubleRow perf mode, contracting
    two 128-chunks per instruction. Accuracy is recovered with a 3-term
    residual split: x = xh + xl, W = wh + wl (each fp8, residuals stored at
    the same scale as the parent so PSUM accumulation needs no rescaling),
    and q ~= wh.xh + wh.xl + wl.xh (the wl.xl term is ~eps^2 and dropped).
    Combined projection error ~0.1%, below the bf16 storage floor.
  - Scales: x*32, W*1024 => PSUM carries q,k,v scaled by 2^15. The scale is
    carried through rope (cos/sin tables unscaled), cancelled in softmax by
    folding 2^-30 into the Exp activation scale, and cancelled in the output
    by setting the appended V-ones column to 2^15 (numerator and denominator
    of the softmax-normalized AV both carry 2^15, so the ratio is exact).
  - scores computed transposed, S^T[k, q], so softmax needs no transposes;
    exp without max-subtraction (logits are ~N(0,0.8)).
  - Diagonal score chunks are narrowed: for diagonal offset doff only query
    columns >= 128*doff survive the causal mask, so the score matmul streams
    512-128*doff columns and only the first 128 of them need the triangular
    mask multiply (one shared [128,128] lower-triangle pattern).
  - softmax denominator folded into the AV matmul by appending the ones
    column to V (V_aug [k, 129]); a per-partition reciprocal-scale
    normalizes at the end.
"""

import sys

if "/opt/trn_rl_repo" not in sys.path:
    sys.path.insert(0, "/opt/trn_rl_repo")

import numpy as np
import ml_dtypes

import concourse.mybir as mybir
import concourse.tile as tile
from concourse import bacc
from concourse.masks import make_identity
from concourse.bass_utils import run_bass_kernel_spmd

BF16 = mybir.dt.bfloat16
FP8 = mybir.dt.float8e4
F32 = mybir.dt.float32
E4 = ml_dtypes.float8_e4m3
P = 128
B, T, C, D = 4, 4096, 2048, 128
CC = C // P          # 16 contraction chunks
NCP = CC // 2        # 8 DoubleRow contraction pair-chunks
TQ = T // 2          # 2048 own query rows per core
NT = T // 512        # 8 sequence tiles
SCALE = float(D) ** -0.5
ROPE_BASE = 10000.0
SX = 32.0            # fp8 scale on x
SW = 1024.0          # fp8 scale on W
SPS = SX * SW        # 2^15: scale carried by q,k,v through rope and V
DR = mybir.MatmulPerfMode.DoubleRow

# query-tile ownership (global 512-row tile indices)
JOBS_H0 = (0, 2, 4, 7)   # needs K/V tiles 0..7
JOBS_H1 = (1, 3, 5, 6)   # needs K/V tiles 0..6 only

_NC_CACHE = None


def _build_nc():
    nc = bacc.Bacc("TRN2", target_bir_lowering=False, debug=False, num_devices=8)

    xTh = nc.declare_dram_parameter("xTh", [C, T], FP8, isOutput=False)
    xTl = nc.declare_dram_parameter("xTl", [C, T], FP8, isOutput=False)
    wqh = nc.declare_dram_parameter("wqh", [P, CC * D], FP8, isOutput=False)
    wql = nc.declare_dram_parameter("wql", [P, CC * D], FP8, isOutput=False)
    wkh = nc.declare_dram_parameter("wkh", [P, CC * D], FP8, isOutput=False)
    wkl = nc.declare_dram_parameter("wkl", [P, CC * D], FP8, isOutput=False)
    wvh = nc.declare_dram_parameter("wvh", [P, CC * D], FP8, isOutput=False)
    wvl = nc.declare_dram_parameter("wvl", [P, CC * D], FP8, isOutput=False)
    cosT = nc.declare_dram_parameter("cosT", [D, T], BF16, isOutput=False)
    sinT = nc.declare_dram_parameter("sinT", [D, T], BF16, isOutput=False)
    bq = nc.declare_dram_parameter("bq", [D, 1], F32, isOutput=False)
    bk = nc.declare_dram_parameter("bk", [D, 1], F32, isOutput=False)
    bv = nc.declare_dram_parameter("bv", [D, 1], F32, isOutput=False)
    tri = nc.declare_dram_parameter("tri", [P, P], BF16, isOutput=False)
    flag = nc.declare_dram_parameter("flag", [1, 1], mybir.dt.int32, isOutput=False)
    out = nc.declare_dram_parameter("out", [TQ, D], F32, isOutput=True)

    xTh_r = xTh.rearrange("(cc p) t -> p cc t", p=P)
    xTl_r = xTl.rearrange("(cc p) t -> p cc t", p=P)

    with tile.TileContext(nc) as tc:
        with (
            tc.tile_pool(name="big", bufs=1) as big,
            tc.tile_pool(name="xh", bufs=3) as xhp,
            tc.tile_pool(name="xl", bufs=3) as xlp,
            tc.tile_pool(name="work", bufs=6) as work,
            tc.tile_pool(name="outp", bufs=6) as outp,
            tc.tile_pool(name="ps", bufs=4, space="PSUM") as ps,
            tc.tile_pool(name="acc", bufs=1, space="PSUM") as accp,
        ):
            # ---- persistent tiles ----
            w_sb = {n: big.tile([P, CC, D], FP8, name=f"w_{n}")
                    for n in ("qh", "ql", "kh", "kl", "vh", "vl")}
            flag_sb = big.tile([1, 1], mybir.dt.int32, name="flag_sb")
            bq_sb = big.tile([P, 1], F32, name="bq_sb")
            bk_sb = big.tile([P, 1], F32, name="bk_sb")
            bv_sb = big.tile([P, 1], F32, name="bv_sb")

            kTs = [big.tile([P, 512], BF16, tag=f"kT{t}", name=f"kT{t}")
                   for t in range(NT)]
            qTs = [big.tile([P, 512], BF16, tag=f"qT{j}", name=f"qT{j}")
                   for j in range(4)]
            vAs = [big.tile([P, 4, D + 1], BF16, tag=f"vA{t}", name=f"vA{t}")
                   for t in range(NT)]
            cos_sb = big.tile([P, T], BF16, name="cos_sb")
            sin_sb = big.tile([P, T], BF16, name="sin_sb")
            tri_sb = big.tile([P, P], BF16, name="tri_sb")
            ident = big.tile([P, P], BF16, name="ident")
            H = D // 2

            # ---- pre-If staging: only what the first K-projection needs ----
            nc.scalar.dma_start(flag_sb[:], flag[:])
            w_r = {"qh": wqh, "ql": wql, "kh": wkh, "kl": wkl,
                   "vh": wvh, "vl": wvl}
            nc.scalar.dma_start(
                w_sb["kh"][:], w_r["kh"].rearrange("p (cc d) -> p cc d", d=D))
            xt0h = xhp.tile([P, CC, 512], FP8, tag="xh")
            xt0l = xlp.tile([P, CC, 512], FP8, tag="xl")
            for g in range(4):
                nc.sync.dma_start(xt0h[:, 4 * g:4 * (g + 1), :],
                                  xTh_r[:, 4 * g:4 * (g + 1), 0:512])
            fv = nc.values_load(flag_sb[0:1, 0:1].to_broadcast((1, 1)))

            def load_consts(own_q0):
                """Branch-local constant loads, critical-path first."""
                for g in range(4):
                    nc.scalar.dma_start(xt0l[:, 4 * g:4 * (g + 1), :],
                                        xTl_r[:, 4 * g:4 * (g + 1), 0:512])
                nc.scalar.dma_start(
                    w_sb["kl"][:],
                    w_r["kl"].rearrange("p (cc d) -> p cc d", d=D))
                nc.scalar.dma_start(bk_sb[:], bk[:])
                nc.scalar.dma_start(cos_sb[:, 0:512], cosT[:, 0:512])
                nc.scalar.dma_start(sin_sb[:, 0:512], sinT[:, 0:512])
                # q weights before v when tile 0's job needs q immediately
                order = ("qh", "ql", "vh", "vl") if own_q0 else \
                        ("vh", "vl", "qh", "ql")
                for n in order:
                    nc.scalar.dma_start(
                        w_sb[n][:],
                        w_r[n].rearrange("p (cc d) -> p cc d", d=D))
                nc.scalar.dma_start(bv_sb[:], bv[:])
                nc.scalar.dma_start(bq_sb[:], bq[:])
                nc.scalar.dma_start(tri_sb[:], tri[:])
                make_identity(nc, ident[:])
                nc.scalar.dma_start(cos_sb[:, 512:T], cosT[:, 512:T])
                nc.scalar.dma_start(sin_sb[:, 512:T], sinT[:, 512:T])
                for t in range(NT):
                    nc.vector.memset(vAs[t][:, :, D], SPS)

            def proj(wh, wl, b_sb, xh_t, xl_t):
                """3-term fp8 DoubleRow projection; returns bf16 [P,512]
                scaled by SPS (plus scaled bias)."""
                pp = ps.tile([P, 512], F32, tag="ps")
                k = 0
                for rhs_t, lhs_t in ((xh_t, wh), (xl_t, wh), (xh_t, wl)):
                    for cp in range(NCP):
                        nc.tensor.matmul(
                            pp[:], lhs_t[:, 2 * cp:2 * cp + 2],
                            rhs_t[:, 2 * cp:2 * cp + 2, :],
                            start=(k == 0), stop=(k == 3 * NCP - 1),
                            perf_mode=DR)
                        k += 1
                raw = work.tile([P, 512], BF16, tag="prj")
                nc.vector.tensor_scalar_add(raw[:], pp[:], b_sb[:])
                return raw

            def rope(dst, raw, sl):
                tmp = work.tile([P, 512], BF16, tag="rtmp")
                nc.sync.dma_start(tmp[0:H, :], raw[H:P, :])
                nc.sync.dma_start(tmp[H:P, :], raw[0:H, :])
                nc.vector.tensor_mul(tmp[:], tmp[:], sin_sb[:, sl])
                nc.vector.tensor_mul(dst[:], raw[:], cos_sb[:, sl])
                nc.vector.tensor_add(dst[:], dst[:], tmp[:])

            def attention_job(tj, qslot, ol0):
                """Causal attention for query tile tj (columns of qTs[qslot]),
                writing output rows [ol0, ol0+512)."""
                accs = [accp.tile([P, D + 1], F32, tag=f"acc{j}",
                                  name=f"acc_{tj}_{j}")[:]
                        for j in range(4)]
                for kc in range(4 * tj + 4):
                    doff = kc - 4 * tj
                    qoff = max(doff, 0) * P
                    width = 512 - qoff
                    ps_s = ps.tile([P, width], F32, tag="ps")
                    nc.tensor.matmul(ps_s[:],
                                     kTs[kc // 4][:, (kc % 4) * P:
                                                  (kc % 4 + 1) * P],
                                     qTs[qslot][:, qoff:512],
                                     start=True, stop=True)
                    ex = work.tile([P, width], BF16, tag="expP")
                    nc.scalar.activation(ex[:], ps_s[:],
                                         mybir.ActivationFunctionType.Exp,
                                         scale=SCALE / (SPS * SPS))
                    if doff >= 0:
                        nc.vector.tensor_mul(ex[:, 0:P], ex[:, 0:P], tri_sb[:])
                    for j in range(max(doff, 0), 4):
                        nc.tensor.matmul(
                            accs[j], ex[:, j * P - qoff:(j + 1) * P - qoff],
                            vAs[kc // 4][:, kc % 4],
                            start=(kc == 0), stop=(kc == 4 * tj + j))
                for j in range(4):
                    rcp = outp.tile([P, 1], F32, tag="rcp")
                    nc.vector.reciprocal(rcp[:], accs[j][:, D:D + 1])
                    ob = outp.tile([P, D], F32, tag="ob")
                    nc.vector.tensor_scalar_mul(ob[:], accs[j][:, 0:D], rcp[:])
                    nc.sync.dma_start(
                        out[ol0 + j * P: ol0 + (j + 1) * P, :], ob[:])

            def branch(jobs, n_kv):
                asc = sorted(jobs)
                load_consts(own_q0=(0 in jobs))
                for tt in range(n_kv):
                    if tt == 0:
                        xh_t, xl_t = xt0h, xt0l
                    else:
                        xh_t = xhp.tile([P, CC, 512], FP8, tag="xh")
                        xl_t = xlp.tile([P, CC, 512], FP8, tag="xl")
                        sl = slice(tt * 512, (tt + 1) * 512)
                        for g in range(2):
                            nc.sync.dma_start(
                                xh_t[:, 8 * g:8 * (g + 1), :],
                                xTh_r[:, 8 * g:8 * (g + 1), sl])
                            nc.scalar.dma_start(
                                xl_t[:, 8 * g:8 * (g + 1), :],
                                xTl_r[:, 8 * g:8 * (g + 1), sl])
                    sl = slice(tt * 512, (tt + 1) * 512)
                    kraw = proj(w_sb["kh"], w_sb["kl"], bk_sb, xh_t, xl_t)
                    rope(kTs[tt], kraw, sl)
                    if tt in jobs:
                        qraw = proj(w_sb["qh"], w_sb["ql"], bq_sb, xh_t, xl_t)
                        rope(qTs[asc.index(tt)], qraw, sl)
                    vraw = proj(w_sb["vh"], w_sb["vl"], bv_sb, xh_t, xl_t)
                    for kk in range(4):
                        ps_t = ps.tile([P, P], BF16, tag="ps")
                        nc.tensor.transpose(ps_t[:],
                                            vraw[:, kk * P:(kk + 1) * P],
                                            ident[:])
                        nc.vector.tensor_copy(vAs[tt][:, kk, 0:D], ps_t[:])
                    if tt in jobs:
                        qi = asc.index(tt)
                        attention_job(tt, qi, qi * 512)

            with tc.If(fv < 1) as cmp:
                branch(JOBS_H0, 8)
            with cmp.Else():
                branch(JOBS_H1, 7)

    nc.compile()
    return nc


def _get_nc():
    global _NC_CACHE
    if _NC_CACHE is None:
        _NC_CACHE = _build_nc()
    return _NC_CACHE


def _own_rows(h):
    jobs = JOBS_H0 if h == 0 else JOBS_H1
    return np.concatenate([np.arange(tj * 512, (tj + 1) * 512)
                           for tj in sorted(jobs)])


def _fp8_split(a, scale):
    """a*scale = hi + lo with hi, lo float8_e4m3 (lo at the same scale)."""
    s = np.asarray(a, np.float32) * scale
    hi = s.astype(E4)
    lo = (s - hi.astype(np.float32)).astype(E4)
    return hi, lo


def _prep_in_maps(x, Wq, Wk, Wv, bq, bk, bv):
    x = np.asarray(x, np.float32)
    bf = ml_dtypes.bfloat16

    # rope tables (rotate-half convention), pre-signed sin
    half = D // 2
    inv = 1.0 / (ROPE_BASE ** (np.arange(half, dtype=np.float32) / half))
    ang = np.arange(T, dtype=np.float32)[:, None] * inv[None, :]       # [T, 64]
    cosT = np.concatenate([np.cos(ang), np.cos(ang)], 1).T.astype(bf)  # [128,T]
    sinT = np.concatenate([-np.sin(ang), np.sin(ang)], 1).T.astype(bf)

    # single [128,128] lower-triangle mask pattern: tri[k, q'] = (k <= q')
    k_idx = np.arange(P)[:, None]
    q_idx = np.arange(P)[None, :]
    tri = (k_idx <= q_idx).astype(bf)

    def _wP(W):
        # [D, C] -> [C, D] -> [p, cc, d] -> [P, CC*D], fp8 hi/lo at scale SW
        wT = np.asarray(W, np.float32).T.reshape(CC, P, D).transpose(1, 0, 2)
        wT = np.ascontiguousarray(wT.reshape(P, CC * D))
        return _fp8_split(wT, SW)

    wqh_a, wql_a = _wP(Wq)
    wkh_a, wkl_a = _wP(Wk)
    wvh_a, wvl_a = _wP(Wv)

    def _b(b):
        return np.ascontiguousarray(
            np.asarray(b, np.float32).reshape(D, 1) * SPS)

    bq_a, bk_a, bv_a = _b(bq), _b(bk), _b(bv)

    x_cache = {}
    in_maps = []
    for c in range(8):
        b, h = c // 2, c % 2
        if b not in x_cache:
            xT_b = np.ascontiguousarray(x[b].T)        # [C, T] f32
            x_cache[b] = _fp8_split(xT_b, SX)
        xh_b, xl_b = x_cache[b]
        in_maps.append({
            "xTh": xh_b, "xTl": xl_b,
            "wqh": wqh_a, "wql": wql_a,
            "wkh": wkh_a, "wkl": wkl_a,
            "wvh": wvh_a, "wvl": wvl_a,
            "cosT": cosT, "sinT": sinT,
            "bq": bq_a, "bk": bk_a, "bv": bv_a,
            "tri": tri,
            "flag": np.array([[h]], np.int32),
        })

    return in_maps


def kernel(x, Wq, Wk, Wv, bq, bk, bv):
    nc = _get_nc()
    in_maps = _prep_in_maps(x, Wq, Wk, Wv, bq, bk, bv)
    res = run_bass_kernel_spmd(nc, in_maps, core_ids=list(range(8)))

    out = np.empty((B, T, D), np.float32)
    for c in range(8):
        b, h = c // 2, c % 2
        out[b, _own_rows(h)] = res.results[c]["out"]
    return out


# revision 13
# speedup vs baseline: 1.4012x; 1.2330x over previous
"""Single-head causal attention with RoPE on 8 TRN2 NeuronCores.

Problem: B=4, T=4096, C=2048, D=128 (fp32 reference).
  q/k/v = x @ W{q,k,v}.T + b ; rope(q), rope(k); causal softmax(q k^T / sqrt(D)) @ v

Sharding: core c -> batch b = c//2, sequence-half h = c%2 with a load-balanced
split of 512-row query tiles: h=0 owns tiles {0,2,4,7}, h=1 owns {1,3,5,6}.
h=1 needs K/V only for tiles 0..6, so it skips tile 7's x load and K/V
projection entirely; h=0 carries 8 K/V tiles but 4 fewer score chunks, which
roughly balances the cores. K/V are computed for the needed prefix on both
cores of a pair (duplicated; no collectives, so each core simulates alone).

One SPMD graph for all 8 cores; the structural difference between the two
halves is a runtime If() on a per-core flag input.

Kernel math:
  - Projections run on fp8 (float8e4) with DoubleRow perf mode, contracting
    two 128-chunks per instruction. Accuracy is recovered with a 3-term
    residual split: x = xh + xl, W = wh + wl (each fp8, residuals stored at
    the same scale as the parent so PSUM accumulation needs no rescaling),
    and q ~= wh.xh + wh.xl + wl.xh (the wl.xl term is ~eps^2 and dropped).
    Combined projection error ~0.1%, below the bf16 storage floor.
  - Scales: x*32, W*1024 => PSUM carries q,k,v scaled by 2^15. The scale is
    carried through rope (cos/sin tables unscaled), cancelled in softmax by
    folding 2^-30 into the Exp activation scale, and cancelled in the output
    by setting the appended V-ones column to 2^15 (numerator and denominator
    of the softmax-normalized AV both carry 2^15, so the ratio is exact).
  - scores computed transposed, S^T[k, q], so softmax needs no transposes;
    exp without max-subtraction (logits are ~N(0,0.8)).
  - Diagonal score chunks are narrowed: for diagonal offset doff only query
    columns >= 128*doff survive the causal mask, so the score matmul streams
    512-128*doff columns and only the first 128 of them need the triangular
    mask multiply (one shared [128,128] lower-triangle pattern).
  - softmax denominator folded into the AV matmul by appending the ones
    column to V (V_aug [k, 129]); a per-partition reciprocal-scale
    normalizes at the end.
"""

import sys

if "/opt/trn_rl_repo" not in sys.path:
    sys.path.insert(0, "/opt/trn_rl_repo")

import numpy as np
import ml_dtypes

import concourse.mybir as mybir
import concourse.tile as tile
from concourse import bacc
from concourse.masks import make_identity
from concourse.bass_utils import run_bass_kernel_spmd

BF16 = mybir.dt.bfloat16
FP8 = mybir.dt.float8e4
F32 = mybir.dt.float32
E4 = ml_dtypes.float8_e4m3
P = 128
B, T, C, D = 4, 4096, 2048, 128
CC = C // P          # 16 contraction chunks
NCP = CC // 2        # 8 DoubleRow contraction pair-chunks
TQ = T // 2          # 2048 own query rows per core
NT = T // 512        # 8 sequence tiles
SCALE = float(D) ** -0.5
ROPE_BASE = 10000.0
SX = 32.0            # fp8 scale on x
SW = 1024.0          # fp8 scale on W
SPS = SX * SW        # 2^15: scale carried by q,k,v through rope and V
DR = mybir.MatmulPerfMode.DoubleRow

# query-tile ownership (global 512-row tile indices)
JOBS_H0 = (0, 2, 4, 7)   # needs K/V tiles 0..7
JOBS_H1 = (1, 3, 5, 6)   # needs K/V tiles 0..6 only
# tile processing order: the big background job's own tile is pulled early so
# its attention chunks can interleave under later tiles' projection work
ORDER_H0 = (0, 1, 7, 2, 3, 4, 5, 6)
ORDER_H1 = (0, 1, 2, 6, 3, 4, 5)
# rope pair shuffle: swap adjacent partitions within each 32-lane quadrant
SHUF_MASK = [i ^ 1 for i in range(32)]

_NC_CACHE = None


def _build_nc():
    nc = bacc.Bacc("TRN2", target_bir_lowering=False, debug=False, num_devices=8)

    xTh = nc.declare_dram_parameter("xTh", [C, T], FP8, isOutput=False)
    xTl = nc.declare_dram_parameter("xTl", [C, T], FP8, isOutput=False)
    wqh = nc.declare_dram_parameter("wqh", [P, CC * D], FP8, isOutput=False)
    wql = nc.declare_dram_parameter("wql", [P, CC * D], FP8, isOutput=False)
    wkh = nc.declare_dram_parameter("wkh", [P, CC * D], FP8, isOutput=False)
    wkl = nc.declare_dram_parameter("wkl", [P, CC * D], FP8, isOutput=False)
    wvh = nc.declare_dram_parameter("wvh", [P, CC * D], FP8, isOutput=False)
    wvl = nc.declare_dram_parameter("wvl", [P, CC * D], FP8, isOutput=False)
    cosT = nc.declare_dram_parameter("cosT", [D, T], BF16, isOutput=False)
    sinT = nc.declare_dram_parameter("sinT", [D, T], BF16, isOutput=False)
    bq = nc.declare_dram_parameter("bq", [D, 1], F32, isOutput=False)
    bk = nc.declare_dram_parameter("bk", [D, 1], F32, isOutput=False)
    bv = nc.declare_dram_parameter("bv", [D, 1], F32, isOutput=False)
    tri = nc.declare_dram_parameter("tri", [P, P], BF16, isOutput=False)
    flag = nc.declare_dram_parameter("flag", [1, 1], mybir.dt.int32, isOutput=False)
    out = nc.declare_dram_parameter("out", [TQ, D], F32, isOutput=True)

    xTh_r = xTh.rearrange("(cc p) t -> p cc t", p=P)
    xTl_r = xTl.rearrange("(cc p) t -> p cc t", p=P)

    with tile.TileContext(nc) as tc:
        with (
            tc.tile_pool(name="big", bufs=1) as big,
            tc.tile_pool(name="xh", bufs=3) as xhp,
            tc.tile_pool(name="xl", bufs=3) as xlp,
            tc.tile_pool(name="work", bufs=6) as work,
            tc.tile_pool(name="outp", bufs=6) as outp,
            tc.tile_pool(name="ps", bufs=3, space="PSUM") as ps,
            tc.tile_pool(name="acc", bufs=1, space="PSUM") as accp,
        ):
            # ---- persistent tiles ----
            w_sb = {n: big.tile([P, CC, D], FP8, name=f"w_{n}")
                    for n in ("qh", "ql", "kh", "kl", "vh", "vl")}
            flag_sb = big.tile([1, 1], mybir.dt.int32, name="flag_sb")
            bq_sb = big.tile([P, 1], F32, name="bq_sb")
            bk_sb = big.tile([P, 1], F32, name="bk_sb")
            bv_sb = big.tile([P, 1], F32, name="bv_sb")

            kTs = [big.tile([P, 512], BF16, tag=f"kT{t}", name=f"kT{t}")
                   for t in range(NT)]
            qTs = [big.tile([P, 512], BF16, tag=f"qT{j}", name=f"qT{j}")
                   for j in range(4)]
            vAs = [big.tile([P, 4, D + 1], BF16, tag=f"vA{t}", name=f"vA{t}")
                   for t in range(NT)]
            cos_sb = big.tile([P, T], BF16, name="cos_sb")
            sin_sb = big.tile([P, T], BF16, name="sin_sb")
            tri_sb = big.tile([P, P], BF16, name="tri_sb")
            ident = big.tile([P, P], BF16, name="ident")
            H = D // 2

            # ---- pre-If staging: only what the first K-projection needs ----
            nc.scalar.dma_start(flag_sb[:], flag[:])
            w_r = {"qh": wqh, "ql": wql, "kh": wkh, "kl": wkl,
                   "vh": wvh, "vl": wvl}
            nc.scalar.dma_start(
                w_sb["kh"][:], w_r["kh"].rearrange("p (cc d) -> p cc d", d=D))
            xt0h = xhp.tile([P, CC, 512], FP8, tag="xh")
            xt0l = xlp.tile([P, CC, 512], FP8, tag="xl")
            for g in range(4):
                nc.sync.dma_start(xt0h[:, 4 * g:4 * (g + 1), :],
                                  xTh_r[:, 4 * g:4 * (g + 1), 0:512])
            fv = nc.values_load(flag_sb[0:1, 0:1].to_broadcast((1, 1)))

            spill_sb = big.tile([P, 4, D + 1], F32, name="spill_sb")

            def load_consts(own_q0):
                """Branch-local constant loads, critical-path first.
                scalar queue: x0-lo + the weights tile 0's own projections
                need; gpsimd (SWDGE) queue: everything else."""
                for g in range(4):
                    nc.scalar.dma_start(xt0l[:, 4 * g:4 * (g + 1), :],
                                        xTl_r[:, 4 * g:4 * (g + 1), 0:512])
                first = ("qh", "ql") if own_q0 else ("vh", "vl")
                rest = ("vh", "vl") if own_q0 else ("qh", "ql")
                for n in first:
                    nc.scalar.dma_start(
                        w_sb[n][:],
                        w_r[n].rearrange("p (cc d) -> p cc d", d=D))
                nc.scalar.dma_start(cos_sb[:, 0:512], cosT[:, 0:512])
                nc.scalar.dma_start(sin_sb[:, 0:512], sinT[:, 0:512])
                nc.gpsimd.dma_start(
                    w_sb["kl"][:],
                    w_r["kl"].rearrange("p (cc d) -> p cc d", d=D))
                nc.gpsimd.dma_start(bk_sb[:], bk[:])
                nc.gpsimd.dma_start(tri_sb[:], tri[:])
                for n in rest:
                    nc.gpsimd.dma_start(
                        w_sb[n][:],
                        w_r[n].rearrange("p (cc d) -> p cc d", d=D))
                nc.gpsimd.dma_start(bv_sb[:], bv[:])
                nc.gpsimd.dma_start(bq_sb[:], bq[:])
                make_identity(nc, ident[:])
                for t in range(NT):
                    nc.vector.memset(vAs[t][:, :, D], SPS)

            def branch(jobs, order):
                trace = _drive(jobs, order, None)
                flags = _resolve(trace)
                _drive(jobs, order, flags)

            def _resolve(trace):
                start, stop, flushjs, finishjs = {}, {}, {}, {}
                segn, cur = {}, {}
                for ev in trace:
                    kind, tj = ev[0], ev[1]
                    if kind == "acq":
                        segn[tj] = segn.get(tj, -1) + 1
                        cur[tj] = {}
                    elif kind == "av":
                        _, _, kc, j = ev
                        cur[tj].setdefault(j, []).append((tj, kc, j))
                    elif kind in ("flush", "finish"):
                        js = sorted(cur.get(tj, {}))
                        if kind == "flush":
                            flushjs[(tj, segn[tj])] = js
                        else:
                            finishjs[tj] = js
                        for keys in cur.get(tj, {}).values():
                            start[keys[0]] = True
                            stop[keys[-1]] = True
                        cur[tj] = {}
                return {"start": start, "stop": stop,
                        "flushjs": flushjs, "finishjs": finishjs}

            def _drive(jobs, order, flags):
                emit = flags is not None
                asc = sorted(jobs)
                bg = max(jobs)
                trace = []
                remaining = {tj: list(range(4 * tj + 4)) for tj in jobs}
                navs = {tj: 0 for tj in jobs}
                totavs = {tj: sum(4 - max(kc - 4 * tj, 0)
                                  for kc in range(4 * tj + 4)) for tj in jobs}
                kq_pt, v_pt, q_pt = {}, {}, {}
                pt = [0]
                active = [None]
                accs = {}
                segn = {}
                spill_init = set()
                spilled = set()
                pend = []

                def ready(tj):
                    if tj not in q_pt or pt[0] < q_pt[tj]:
                        return []
                    out = []
                    for kc in remaining[tj]:
                        t = kc // 4
                        if (t in kq_pt and pt[0] >= kq_pt[t]
                                and t in v_pt and pt[0] >= v_pt[t]):
                            out.append(kc)
                    return out

                def emit_avs(entry):
                    tj, kc, ex, qoff = entry
                    doff = kc - 4 * tj
                    for j in range(max(doff, 0), 4):
                        key = (tj, kc, j)
                        trace.append(("av", tj, kc, j))
                        navs[tj] += 1
                        if emit:
                            nc.tensor.matmul(
                                accs[j], ex[:, j * P - qoff:
                                            (j + 1) * P - qoff],
                                vAs[kc // 4][:, kc % 4],
                                start=flags["start"].get(key, False),
                                stop=flags["stop"].get(key, False))
                    if navs[tj] == totavs[tj]:
                        finish(tj)

                def finish(tj):
                    trace.append(("finish", tj))
                    active[0] = None
                    if not emit:
                        return
                    js = flags["finishjs"][tj]
                    ol0 = asc.index(tj) * 512
                    for j in range(4):
                        if (tj, j) in spill_init:
                            sl_j = spill_sb[:, j, :]
                            if j in js:
                                nc.vector.tensor_add(sl_j, sl_j, accs[j])
                            num = spill_sb[:, j, 0:D]
                            den = spill_sb[:, j, D:D + 1]
                        else:
                            num = accs[j][:, 0:D]
                            den = accs[j][:, D:D + 1]
                        rcp = outp.tile([P, 1], F32, tag="rcp")
                        nc.vector.reciprocal(rcp[:], den)
                        ob = outp.tile([P, D], F32, tag="ob")
                        nc.vector.tensor_scalar_mul(ob[:], num, rcp[:])
                        nc.sync.dma_start(
                            out[ol0 + j * P: ol0 + (j + 1) * P, :], ob[:])

                def flush(tj):
                    trace.append(("flush", tj))
                    spilled.add(tj)
                    if not emit:
                        return
                    for j in flags["flushjs"][(tj, segn[tj])]:
                        sl_j = spill_sb[:, j, :]
                        if (tj, j) in spill_init:
                            nc.vector.tensor_add(sl_j, sl_j, accs[j])
                        else:
                            nc.vector.tensor_copy(sl_j, accs[j])
                            spill_init.add((tj, j))

                def acquire(tj):
                    trace.append(("acq", tj))
                    segn[tj] = segn.get(tj, -1) + 1
                    active[0] = tj
                    if emit:
                        for j in range(4):
                            accs[j] = accp.tile([P, D + 1], F32,
                                                tag=f"acc{j}",
                                                name=f"acc_{tj}_{segn[tj]}_{j}")[:]

                def emit_score(tj, kc):
                    doff = kc - 4 * tj
                    qoff = max(doff, 0) * P
                    ex = None
                    if emit:
                        width = 512 - qoff
                        ps_s = ps.tile([P, width], F32, tag="ps")
                        nc.tensor.matmul(
                            ps_s[:],
                            kTs[kc // 4][:, (kc % 4) * P:(kc % 4 + 1) * P],
                            qTs[asc.index(tj)][:, qoff:512],
                            start=True, stop=True)
                        ex = work.tile([P, width], BF16, tag="expP")
                        nc.scalar.activation(
                            ex[:], ps_s[:], mybir.ActivationFunctionType.Exp,
                            scale=SCALE / (SPS * SPS))
                        if doff >= 0:
                            nc.vector.tensor_mul(ex[:, 0:P], ex[:, 0:P],
                                                 tri_sb[:])
                    remaining[tj].remove(kc)
                    pend.append((tj, kc, ex, qoff))

                def point(force=False):
                    pt[0] += 1
                    if pend and (force or len(pend) >= 2):
                        emit_avs(pend.pop(0))
                    cand = None
                    if active[0] is not None and ready(active[0]):
                        cand = active[0]
                    else:
                        for tj in asc:
                            if tj != bg and remaining[tj] and ready(tj):
                                cand = tj
                                break
                        if cand is None and remaining[bg] and ready(bg):
                            cand = bg
                    if cand is None:
                        return
                    if cand != active[0]:
                        if active[0] is not None:
                            old = active[0]
                            while pend:
                                emit_avs(pend.pop(0))
                            if active[0] == old:  # not finished by drain
                                flush(old)
                        acquire(cand)
                    rd = ready(cand)
                    if rd:
                        emit_score(cand, min(rd))

                def proj(wname, b_sb, xh_t, xl_t):
                    pp = None
                    if emit:
                        pp = ps.tile([P, 512], F32, tag="ps")
                    k = 0
                    for rhs_t, lhs_t in ((xh_t, w_sb[wname + "h"]),
                                         (xl_t, w_sb[wname + "h"]),
                                         (xh_t, w_sb[wname + "l"])):
                        for cp in range(NCP):
                            if emit:
                                nc.tensor.matmul(
                                    pp[:], lhs_t[:, 2 * cp:2 * cp + 2],
                                    rhs_t[:, 2 * cp:2 * cp + 2, :],
                                    start=(k == 0), stop=(k == 3 * NCP - 1),
                                    perf_mode=DR)
                            k += 1
                            if k % 2 == 0:
                                point()
                    raw = None
                    if emit:
                        raw = work.tile([P, 512], BF16, tag="prj")
                        nc.vector.tensor_scalar_add(raw[:], pp[:], b_sb[:])
                    return raw

                def rope(dst, raw, sl):
                    if not emit:
                        return
                    tmp = work.tile([P, 512], BF16, tag="rtmp")
                    nc.vector.stream_shuffle(tmp[:], raw[:], SHUF_MASK)
                    nc.vector.tensor_mul(tmp[:], tmp[:], sin_sb[:, sl])
                    nc.vector.tensor_mul(dst[:], raw[:], cos_sb[:, sl])
                    nc.vector.tensor_add(dst[:], dst[:], tmp[:])

                def emit_x_dma(t, xh_t, xl_t):
                    sl = slice(t * 512, (t + 1) * 512)
                    nc.sync.dma_start(xh_t[:], xTh_r[:, :, sl])
                    nc.sync.dma_start(xl_t[:], xTl_r[:, :, sl])

                xtiles = {0: (xt0h, xt0l)}
                if emit:
                    load_consts(own_q0=(order[0] in jobs))

                for pos, tt in enumerate(order):
                    if emit:
                        # prefetch x and rope tables two tiles ahead
                        for fut in ([order[1], order[2]] if pos == 0
                                    else ([order[pos + 2]]
                                          if pos + 2 < len(order) else [])):
                            xh_t = xhp.tile([P, CC, 512], FP8, tag="xh",
                                            name=f"x{fut}h")
                            xl_t = xlp.tile([P, CC, 512], FP8, tag="xl",
                                            name=f"x{fut}l")
                            xtiles[fut] = (xh_t, xl_t)
                            emit_x_dma(fut, xh_t, xl_t)
                        if pos + 1 < len(order):
                            nxt = order[pos + 1]
                            nsl = slice(nxt * 512, (nxt + 1) * 512)
                            nc.gpsimd.dma_start(cos_sb[:, nsl], cosT[:, nsl])
                            nc.gpsimd.dma_start(sin_sb[:, nsl], sinT[:, nsl])
                    xh_t, xl_t = xtiles.get(tt, (None, None))
                    sl = slice(tt * 512, (tt + 1) * 512)
                    kraw = proj("k", bk_sb, xh_t, xl_t)
                    rope(kTs[tt], kraw, sl)
                    kq_pt[tt] = pt[0] + 4
                    point()
                    if tt in jobs:
                        qraw = proj("q", bq_sb, xh_t, xl_t)
                        rope(qTs[asc.index(tt)], qraw, sl)
                        q_pt[tt] = pt[0] + 4
                        point()
                    vraw = proj("v", bv_sb, xh_t, xl_t)
                    ps_t4 = None
                    if emit:
                        ps_t4 = ps.tile([P, 4, P], BF16, tag="pt", bufs=1)
                    for kk in range(4):
                        if emit:
                            nc.tensor.transpose(ps_t4[:, kk, :],
                                                vraw[:, kk * P:(kk + 1) * P],
                                                ident[:])
                        point()
                    if emit:
                        nc.vector.tensor_copy(vAs[tt][:, :, 0:D], ps_t4[:])
                    v_pt[tt] = pt[0] + 2

                guard = 0
                while any(remaining.values()) or pend:
                    point(force=True)
                    guard += 1
                    assert guard < 10000, "drain did not converge"
                return trace

            with tc.If(fv < 1) as cmp:
                branch(JOBS_H0, ORDER_H0)
            with cmp.Else():
                branch(JOBS_H1, ORDER_H1)

    nc.compile()
    return nc


def _get_nc():
    global _NC_CACHE
    if _NC_CACHE is None:
        _NC_CACHE = _build_nc()
    return _NC_CACHE


def _own_rows(h):
    jobs = JOBS_H0 if h == 0 else JOBS_H1
    return np.concatenate([np.arange(tj * 512, (tj + 1) * 512)
                           for tj in sorted(jobs)])


def _fp8_split(a, scale):
    """a*scale = hi + lo with hi, lo float8_e4m3 (lo at the same scale)."""
    s = np.asarray(a, np.float32) * scale
    hi = s.astype(E4)
    lo = (s - hi.astype(np.float32)).astype(E4)
    return hi, lo


def _prep_in_maps(x, Wq, Wk, Wv, bq, bk, bv):
    x = np.asarray(x, np.float32)
    bf = ml_dtypes.bfloat16

    # rope tables (rotate-half convention) in the pair-interleaved D layout:
    # row 2i carries old row i, row 2i+1 carries old row 64+i, so the
    # rotate-half partner is the adjacent partition (stream_shuffle-able).
    # sin is pre-signed: -sin on even rows, +sin on odd rows.
    half = D // 2
    inv = 1.0 / (ROPE_BASE ** (np.arange(half, dtype=np.float32) / half))
    ang = np.arange(T, dtype=np.float32)[:, None] * inv[None, :]       # [T, 64]
    cosT = np.empty((D, T), np.float32)
    sinT = np.empty((D, T), np.float32)
    cosT[0::2] = cosT[1::2] = np.cos(ang).T
    sinT[0::2] = -np.sin(ang).T
    sinT[1::2] = np.sin(ang).T
    cosT = cosT.astype(bf)
    sinT = sinT.astype(bf)

    # matching permutation of the D axis for Wq/Wk/bq/bk (scores are
    # invariant because q and k share it; V is untouched)
    perm = np.empty(D, np.int64)
    perm[0::2] = np.arange(half)
    perm[1::2] = half + np.arange(half)

    # single [128,128] lower-triangle mask pattern: tri[k, q'] = (k <= q')
    k_idx = np.arange(P)[:, None]
    q_idx = np.arange(P)[None, :]
    tri = (k_idx <= q_idx).astype(bf)

    def _wP(W):
        # [D, C] -> [C, D] -> [p, cc, d] -> [P, CC*D], fp8 hi/lo at scale SW
        wT = np.asarray(W, np.float32).T.reshape(CC, P, D).transpose(1, 0, 2)
        wT = np.ascontiguousarray(wT.reshape(P, CC * D))
        return _fp8_split(wT, SW)

    wqh_a, wql_a = _wP(np.asarray(Wq)[perm])
    wkh_a, wkl_a = _wP(np.asarray(Wk)[perm])
    wvh_a, wvl_a = _wP(Wv)

    def _b(b, p=None):
        b = np.asarray(b, np.float32)
        if p is not None:
            b = b[p]
        return np.ascontiguousarray(b.reshape(D, 1) * SPS)

    bq_a, bk_a, bv_a = _b(bq, perm), _b(bk, perm), _b(bv)

    x_cache = {}
    in_maps = []
    for c in range(8):
        b, h = c // 2, c % 2
        if b not in x_cache:
            xT_b = np.ascontiguousarray(x[b].T)        # [C, T] f32
            x_cache[b] = _fp8_split(xT_b, SX)
        xh_b, xl_b = x_cache[b]
        in_maps.append({
            "xTh": xh_b, "xTl": xl_b,
            "wqh": wqh_a, "wql": wql_a,
            "wkh": wkh_a, "wkl": wkl_a,
            "wvh": wvh_a, "wvl": wvl_a,
            "cosT": cosT, "sinT": sinT,
            "bq": bq_a, "bk": bk_a, "bv": bv_a,
            "tri": tri,
            "flag": np.array([[h]], np.int32),
        })

    return in_maps


def kernel(x, Wq, Wk, Wv, bq, bk, bv):
    nc = _get_nc()
    in_maps = _prep_in_maps(x, Wq, Wk, Wv, bq, bk, bv)
    res = run_bass_kernel_spmd(nc, in_maps, core_ids=list(range(8)))

    out = np.empty((B, T, D), np.float32)
    for c in range(8):
        b, h = c // 2, c % 2
        out[b, _own_rows(h)] = res.results[c]["out"]
    return out
